# revision 14
# baseline (speedup 1.0000x reference)
"""DistMult edge-scoring kernel for Trainium2 (8 NeuronCores, SPMD).

score[j] = sum_d emb_A[a_idx[j], d] * k[d] * emb_B[b_idx[j], d]
for 9E pairs: E positive edges, 4E head-corrupted, 4E tail-corrupted.

Strategy (v7, dense bf16 streaming + batched vector ops):
- The relation kernel k is folded into the B table on the host
  (Bk = emb_B * k), and both tables are converted to bf16, halving all
  HBM traffic (error ~0.3%, well within the 2e-2 gate).
- Per edge, the 9 pairs share rows: a = A[src] is used by the positive
  and the 4 tail-corrupt pairs; b = Bk[dst] by the positive and the 4
  head-corrupt pairs. The host packs, per edge, the 10 distinct rows
  [a | h0..h3 | b | t0..t3] into one dense per-core stream, so the
  device reads 2560 B/edge instead of 9*512 B — pure sequential HWDGE
  DMA at near-peak bandwidth, no gathers, no gpsimd.
- Edges are split contiguously across the 8 cores (12500 each); 128
  edges per partition-block, up to 14 blocks per DMA group (tapered at
  the pipeline edges to shrink fill/drain). Per group the vector engine
  runs a handful of wide instructions (per-pair STT dispatch overhead
  dominated earlier variants): broadcast multiplies [a,h0..h3]*b
  (positive + head pairs) and [t0..t3]*a, an in-place bf16 add tree
  (tensor_tensor keeps the 16-bit 2x perf mode; tensor_reduce has no
  2x uop) down to 8-wide chunks, and one small fp32 tensor_reduce.
  Scores stream out per group, overlapped with the next group's DMA.
- Host reassembles the three score sections from the per-core tiles.
"""

import numpy as np

# problem constants
N_A = 100000
N_B = 100000
D = 128
E = 100000
NEG = 4
NCORES = 8

P = 128
EPC = E // NCORES            # 12500 edges per core
NBLK = -(-EPC // P)          # 98 blocks of 128 edges
EPAD = NBLK * P              # 12544
ROWS = 2 + 2 * NEG           # a, h0-3, b, t0-3
WBLK = ROWS * D              # 1280 stream columns per block
GB = 14                      # blocks per DMA group
NGRP = NBLK // GB            # 7 groups
NS = 2 * NEG + 1             # 9 scores per edge

_CACHED = {}


def _build_program(repeat=1):
    """One SPMD program for all cores. repeat>1 wraps the body in a
    hardware For_i loop (used only for loop-amplified timing)."""
    import concourse.tile as tile
    from concourse import bacc, mybir

    f32 = mybir.dt.float32
    bf16 = mybir.dt.bfloat16
    mult = mybir.AluOpType.mult
    add = mybir.AluOpType.add

    nc = bacc.Bacc("TRN2", target_bir_lowering=False, debug=False,
                   num_devices=NCORES)
    stream_d = nc.dram_tensor("stream", [P, NBLK * WBLK], bf16,
                              kind="ExternalInput").ap()
    s_out = nc.dram_tensor("scores", [P, NBLK * NS], f32,
                           kind="ExternalOutput").ap()

    with tile.TileContext(nc) as tc:
        with (
            tc.tile_pool(name="in", bufs=3) as in_pool,
            tc.tile_pool(name="prod", bufs=2) as prod_pool,
            tc.tile_pool(name="scores", bufs=2) as s_pool,
        ):
            # Taper the schedule: small chunks at the edges shrink the
            # pipeline fill (first DMA uncovered) and drain (last compute
            # uncovered); big chunks in the middle keep instruction count low.
            sched = [GB // 2, GB // 2] + [GB] * (NGRP - 2) + [GB // 2, GB // 2]

            def body():
                b0 = 0
                for gb in sched:
                    seg = gb * NS
                    T = in_pool.tile([P, GB * WBLK], bf16, tag="in")
                    nc.sync.dma_start(
                        T[:, :gb * WBLK],
                        stream_d[:, b0 * WBLK:(b0 + gb) * WBLK])
                    Tv = T[:, :gb * WBLK].rearrange(
                        "p (k s d) -> p k s d", k=gb, s=ROWS)
                    Pr = prod_pool.tile([P, GB * NS * D], bf16, tag="pr")
                    Pv = Pr[:, :seg * D].rearrange(
                        "p (k s d) -> p k s d", k=gb, s=NS)
                    # positive pair a*b: no broadcast needed, so the
                    # otherwise-idle gpsimd engine computes it concurrently
                    # with the vector engine's two broadcast multiplies.
                    nc.gpsimd.tensor_tensor(
                        out=Pv[:, :, 0:1, :],
                        in0=Tv[:, :, 0:1, :],
                        in1=Tv[:, :, 1 + NEG:2 + NEG, :],
                        op=mult)
                    # head-corrupt: [h0..h3] * b
                    nc.vector.tensor_tensor(
                        out=Pv[:, :, 1:1 + NEG, :],
                        in0=Tv[:, :, 1:1 + NEG, :],
                        in1=Tv[:, :, 1 + NEG:2 + NEG, :].broadcast_to(
                            [P, gb, NEG, D]),
                        op=mult)
                    # tail-corrupt: [t0..t3] * a
                    nc.vector.tensor_tensor(
                        out=Pv[:, :, 1 + NEG:, :],
                        in0=Tv[:, :, 2 + NEG:, :],
                        in1=Tv[:, :, 0:1, :].broadcast_to([P, gb, NEG, D]),
                        op=mult)
                    # segmented reduce. tensor_reduce has no 16-bit 2x uop,
                    # so halve in-place with tensor_tensor adds (which do run
                    # 2x in bf16) down to 8-wide chunks, then one small fp32
                    # tensor_reduce finishes each segment.
                    P3 = Pr[:, :seg * D].rearrange("p (c e) -> p c e", e=D)
                    with nc.allow_low_precision(
                            "bf16 tree partials; fp32 final reduce"):
                        w = D // 2
                        while w >= 4:
                            nc.vector.tensor_tensor(
                                out=P3[:, :, 0:w], in0=P3[:, :, 0:w],
                                in1=P3[:, :, w:2 * w], op=add)
                            w //= 2
                    sc = s_pool.tile([P, GB * NS], f32, tag="sc")
                    nc.vector.tensor_reduce(
                        out=sc[:, :seg], in_=P3[:, :, 0:4],
                        axis=mybir.AxisListType.X, op=add)
                    nc.sync.dma_start(
                        s_out[:, b0 * NS:(b0 + gb) * NS], sc[:, :seg])
                    b0 += gb

            if repeat == 1:
                body()
            else:
                with tc.For_i(0, repeat, 1):
                    body()

    nc.compile()
    return nc


def _host_pack(A16, Bk16, e0, e1, hb, tb):
    """Build the [P, NBLK*WBLK] bf16 stream for one core's edges."""
    pad = EPAD - e0.shape[0]
    if pad:
        e0 = np.concatenate([e0, e0[:pad]])
        e1 = np.concatenate([e1, e1[:pad]])
        hb = np.concatenate([hb, hb[:pad]])
        tb = np.concatenate([tb, tb[:pad]])
    rows = np.empty((EPAD, ROWS, D), A16.dtype)
    rows[:, 0] = A16[e0]
    rows[:, 1:1 + NEG] = A16[hb]
    rows[:, 1 + NEG] = Bk16[e1]
    rows[:, 2 + NEG:] = Bk16[tb]
    stream = rows.reshape(NBLK, P, WBLK).transpose(1, 0, 2)
    return np.ascontiguousarray(stream).reshape(P, NBLK * WBLK)


def kernel(emb_A, emb_B, rel_kernel, edge_pos, head_batch, tail_batch):
    import ml_dtypes
    from concourse.bass_utils import run_bass_kernel_spmd

    bf = ml_dtypes.bfloat16
    A16 = np.asarray(emb_A, dtype=np.float32).astype(bf)
    kv = np.asarray(rel_kernel, dtype=np.float32)[0]
    Bk16 = (np.asarray(emb_B, dtype=np.float32) * kv[None, :]).astype(bf)
    ep = np.asarray(edge_pos)
    hb = np.asarray(head_batch)
    tb = np.asarray(tail_batch)

    in_maps = []
    for c in range(NCORES):
        sl = slice(c * EPC, (c + 1) * EPC)
        in_maps.append({
            "stream": _host_pack(A16, Bk16, ep[0, sl], ep[1, sl],
                                 hb[sl], tb[sl]),
        })

    if _CACHED.get("sig") != "v8":
        _CACHED["nc"] = _build_program()
        _CACHED["sig"] = "v8"
    nc = _CACHED["nc"]
    _CACHED["in_maps"] = in_maps
    _CACHED["plan"] = ("v8",)

    res = run_bass_kernel_spmd(nc, in_maps, core_ids=list(range(NCORES)))
    _CACHED["last_results"] = res

    out = np.empty(9 * E, dtype=np.float32)
    for c in range(NCORES):
        S = np.asarray(res.results[c]["scores"])           # [P, NBLK*9]
        es = S.reshape(P, NBLK, NS).transpose(1, 0, 2).reshape(EPAD, NS)[:EPC]
        out[c * EPC:(c + 1) * EPC] = es[:, 0]
        h0 = E + c * EPC * NEG
        out[h0:h0 + EPC * NEG] = es[:, 1:1 + NEG].reshape(-1)
        t0 = 5 * E + c * EPC * NEG
        out[t0:t0 + EPC * NEG] = es[:, 1 + NEG:].reshape(-1)
    return out
